# revision 1
# baseline (speedup 1.0000x reference)
"""Trainium2 Bass kernel for CrossDecoder kNN-mining margin loss.

Math: reference mines, per query q (both columns of train_ill), the k+1
nearest rows of X (rows = concat of both manifolds, dim 512) and uses the
*distances* from q to its own k nearest neighbours (self excluded) in a
margin loss.  Ranking and values only need, per query row, the top-(k+1)
smallest of  dist(q, j) = |q|^2 + |y_j|^2 - 2 q.y_j ; since |q|^2 is
row-constant we rank by  score(q,j) = 2 q.y_j - |y_j|^2  (descending) and
recover dist = |q|^2 - score on the host.

Device work (SPMD over 8 cores, candidate axis sharded 30000 -> 3750/core):
  - scores for a [128-query, 470-candidate] tile: the idle Scalar engine
    pre-writes -|y|^2 into the PSUM tile, then 4 accumulated K=128 fp16
    matmuls (queries pre-scaled by 2) add 2 q.y on top (start=False).
    The first 16 groups instead open with a start=True K=2 fp16 bias
    matmul: NEFF load clears PSUM has_written bits, and a start=False
    matmul on a cleared bit would overwrite the ACT-written bias.
  - nc.vector.max (top-8, descending) per chunk straight off PSUM.
Each core emits, per query, 8 chunks x top-8 = 64 candidate scores.
Host merges 8 cores x 64 = 512 candidates/row -> exact top-(k+1) w.p. 1
(would only fail if >8 of the true top-11 landed in one 470-wide chunk).
"""

import os
import numpy as np

M_, N_, D_, T_ = 2, 30000, 256, 3000
NCORES = 8
NSHARD = N_ // NCORES          # 3750
FCH = 470                      # candidate chunk width (>=256 keeps fp32r at full
                               # rate; must be EVEN: fp32r ISA requires even
                               # innermost free count on moving operand and dst)
NFC = 8                        # chunks per shard
NPAD = FCH * NFC               # 3760
KD = M_ * D_                   # 512 contraction dim
KCH = KD // 128                # 4 K-chunks
QT = 128                       # queries per tile (PSUM partition dim)
NQ = 6016                      # both query sets packed (6000) padded to 47 tiles
NQT = NQ // QT                 # 47 query tiles
QBLK = 4                       # query tiles per DMA block
NBLK = 12                      # 11 full blocks + one 3-tile block
PAD_SQY = 60000.0              # pad candidates rank last (fits fp16)

_cache = {}


def _build_program():
    import concourse.bass as bass
    import concourse.tile as tile
    from concourse import bacc, mybir

    dt = mybir.dt
    nc = bacc.Bacc(
        "TRN2", target_bir_lowering=False, debug=False, num_devices=NCORES
    )

    xq_d = nc.dram_tensor("xq", [KCH, 128, NQ], dt.float16, kind="ExternalInput")
    xs_d = nc.dram_tensor("xs", [KCH, 128, NPAD], dt.float16, kind="ExternalInput")
    # -|y|^2 bias, broadcast across partitions; the idle Scalar engine
    # copies it into each PSUM tile and the data matmuls (start=False)
    # accumulate on top, keeping the PE at its pure-matmul floor.
    sqyb_d = nc.dram_tensor("sqyb", [128, NPAD], dt.float32, kind="ExternalInput")
    # fp16 hi/lo bias rows + -1 weights for the first-16 "opener" groups
    sqy_d = nc.dram_tensor("sqy", [2, NPAD], dt.float16, kind="ExternalInput")
    neg1_d = nc.dram_tensor("neg1", [2, QT], dt.float16, kind="ExternalInput")
    cand_d = nc.dram_tensor("cand", [NBLK, 128, QBLK * 64], dt.float32,
                            kind="ExternalOutput")

    with tile.TileContext(nc) as tc:
        with (
            tc.tile_pool(name="resident", bufs=1) as res_pool,
            tc.tile_pool(name="xq", bufs=2) as xq_pool,
            tc.tile_pool(name="cand", bufs=2) as cand_pool,
            tc.tile_pool(name="psum", bufs=8, space=bass.MemorySpace.PSUM) as psum_pool,
        ):
            xs_sb = [res_pool.tile([128, NPAD], dt.float16, tag=f"xs{k}", name=f"xs_sb{k}")
                     for k in range(KCH)]
            for k in range(KCH):
                nc.sync.dma_start(out=xs_sb[k][:, :], in_=xs_d[k])
            sqyb_sb = res_pool.tile([128, NPAD], dt.float32, tag="sqyb")
            nc.sync.dma_start(out=sqyb_sb[:, :], in_=sqyb_d[:, :])
            sqy_sb = res_pool.tile([2, NPAD], dt.float16, tag="sqy")
            nc.sync.dma_start(out=sqy_sb[:, :], in_=sqy_d[:, :])
            neg1_sb = res_pool.tile([2, QT], dt.float16, tag="neg1")
            nc.sync.dma_start(out=neg1_sb[:, :], in_=neg1_d[:, :])

            # Pre-bias each PSUM tile with -|y|^2 on the (otherwise idle)
            # Scalar engine, then accumulate the four data matmuls on top
            # with start=False (skip_group_check: the group is opened by the
            # ACT write, which the group checker doesn't know about).
            from concourse import mybir as _mb
            ui = 0
            for blk in range(NBLK):
                q0 = blk * QBLK * QT
                nqt = min(QBLK, NQT - blk * QBLK)
                xq_sb = [xq_pool.tile([128, nqt * QT], dt.float16,
                                      tag=f"xq{k}", name=f"xq_sb{k}")
                         for k in range(KCH)]
                for k in range(KCH):
                    nc.sync.dma_start(out=xq_sb[k][:, :],
                                      in_=xq_d[k, :, q0:q0 + nqt * QT])
                cand_sb = cand_pool.tile([128, nqt * 64], dt.float32, tag="cand")
                for j in range(nqt):
                    for f in range(NFC):
                        ps = psum_pool.tile([128, FCH], dt.float32, tag="ps")
                        # NEFF load clears PSUM has_written bits; a start=False
                        # matmul on a cleared bit OVERWRITES instead of
                        # accumulating, which would discard the ACT-written
                        # bias. The first 16 groups (>= 2 full trips through
                        # the 8 PSUM slots) open with a start=True matmul to
                        # set the bits; afterwards they stay set for the rest
                        # of the kernel and the cheap ACT pre-bias is safe.
                        if ui < 16:
                            nc.tensor.matmul(
                                ps[:, :], lhsT=neg1_sb[:, :],
                                rhs=sqy_sb[:, f * FCH:(f + 1) * FCH],
                                start=True, stop=False,
                            )
                        else:
                            nc.scalar.activation(
                                ps[:, :], sqyb_sb[:, f * FCH:(f + 1) * FCH],
                                _mb.ActivationFunctionType.Copy,
                            )
                        ui += 1
                        for k in range(KCH):
                            nc.tensor.matmul(
                                ps[:, :],
                                lhsT=xq_sb[k][:, j * QT:(j + 1) * QT],
                                rhs=xs_sb[k][:, f * FCH:(f + 1) * FCH],
                                start=False,
                                stop=(k == KCH - 1),
                                skip_group_check=True,
                            )
                        o = j * 64 + f * 8
                        nc.vector.max(cand_sb[:, o:o + 8], ps[:, :])
                nc.sync.dma_start(out=cand_d[blk, :, :nqt * 64],
                                  in_=cand_sb[:, :])

    nc.compile()
    return nc


def _get_program():
    if "nc" not in _cache:
        _cache["nc"] = _build_program()
    return _cache["nc"]


def _prep_inputs(X, left, right):
    """X: [N, 512] fp32. Returns (shared xq map entries, per-core xs/sqy)."""
    q_idx = np.concatenate([right, left, np.zeros(NQ - 2 * T_, np.int64)])
    Xq = (2.0 * X[q_idx]).astype(np.float16)
    Xq[2 * T_:] = 0.0
    xq_in = np.ascontiguousarray(Xq.T.reshape(KCH, 128, NQ))

    per_core = []
    for corei in range(NCORES):
        shard = X[corei * NSHARD:(corei + 1) * NSHARD]          # [3750, 512]
        xs = np.zeros((KD, NPAD), np.float16)
        xs[:, :NSHARD] = shard.T.astype(np.float16)
        sqy = np.full(NPAD, PAD_SQY, np.float32)
        sqy[:NSHARD] = (shard.astype(np.float64) ** 2).sum(1).astype(np.float32)
        sqy_hi = sqy.astype(np.float16)
        sqy_lo = (sqy - sqy_hi.astype(np.float32)).astype(np.float16)
        per_core.append({
            "xq": xq_in,
            "xs": np.ascontiguousarray(xs.reshape(KCH, 128, NPAD)),
            "sqyb": np.ascontiguousarray(np.broadcast_to(-sqy, (128, NPAD))).astype(np.float32),
            "sqy": np.stack([sqy_hi, sqy_lo]),
            "neg1": np.full((2, QT), -1.0, np.float16),
        })
    return per_core


def _mine_scores(in_maps, trace=False):
    from concourse.bass_utils import run_bass_kernel_spmd

    nc = _get_program()
    try:
        res = run_bass_kernel_spmd(nc, in_maps, list(range(NCORES)), trace=trace)
    except Exception:
        if not trace:
            raise
        res = run_bass_kernel_spmd(nc, in_maps, list(range(NCORES)), trace=False)
    _cache["last_result"] = res
    # per-core cand: [NBLK, 128, QBLK*64] -> [NQ, 64]
    cores = []
    for i in range(NCORES):
        c = res.results[i]["cand"].reshape(NBLK, 128, QBLK, 64)
        cores.append(c.transpose(0, 2, 1, 3).reshape(NBLK * QBLK * 128, 64)[:NQ])
    return np.concatenate(cores, axis=1)                         # [NQ, 512]


def kernel(outlayer, c, train_ill, k):
    k = int(k)
    outlayer = np.asarray(outlayer, np.float32)
    train_ill = np.asarray(train_ill)
    X = np.ascontiguousarray(
        outlayer.transpose(1, 0, 2).reshape(N_, KD)).astype(np.float32)
    left = train_ill[:, 0].astype(np.int64)
    right = train_ill[:, 1].astype(np.int64)

    in_maps = _prep_inputs(X, left, right)
    scores = _mine_scores(in_maps, trace=bool(int(os.environ.get("KNN_TRACE", "0"))))

    # top-(k+1) scores (descending) per query row; row 0 is the self match.
    nkeep = k + 1
    part = np.partition(scores, scores.shape[1] - nkeep, axis=1)[:, -nkeep:]
    top = np.sort(part, axis=1)[:, ::-1]                         # [NQ, k+1]

    X64 = X.astype(np.float64)
    sq = (X64 ** 2).sum(1)                                       # [N]

    s_right = top[:T_]                                           # mining of right idx
    s_left = top[T_:2 * T_]                                      # mining of left idx

    # B[i, j] = dist(q_i, j-th NN of q_i) = |q_i|^2 - score, self (col 0) dropped
    B2 = sq[right][:, None] - s_right[:, 1:].astype(np.float64)
    B1 = sq[left][:, None] - s_left[:, 1:].astype(np.float64)

    D = ((X64[left] - X64[right]) ** 2).sum(1) + 1.0             # [t]
    L1 = np.maximum(D[:, None] - B1, 0.0)
    L2 = np.maximum(D[:, None] - B2, 0.0)
    loss = (L1.mean() + L2.mean()) / 2.0
    return np.asarray(loss, dtype=np.float32)



# revision 2
# speedup vs baseline: 1.4447x; 1.4447x over previous
"""Trainium2 Bass kernel for CrossDecoder kNN-mining margin loss (fp8 version).

Strategy vs the fp16 baseline (336us): the PE work drops 2x by mining in fp8
E4M3 with perf_mode=DoubleRow (2 fp8 weights/cell, K=256 per matmul).  fp8
scores are too noisy (sigma ~3.3) to use as distance *values*, so the device
returns per-16-candidate-chunk maxima instead: the host selects the top
chunks per query by noisy score (position identifies the candidates), then
rescores those ~512 candidates per query exactly in fp32/fp64 and rebuilds
the exact top-(k+1) distances.  Chunk selection has a huge noise margin
(score gap between the 11th candidate and the 32nd chunk ~ 5 sigma).

Device work (SPMD over 8 cores, candidates sharded 30000 -> 3750+90pad):
  score(q,j) = sum_d 2 q_d y_jd  (510 of 512 data dims, fp8)
             + 32*b1_j + b2_j    (2 fp8 bias rows ~= -(|y_j|^2 - 512))
  as 2 DoubleRow matmuls (K=2x256=512 rows = 510 data + 2 bias) per
  [128-query, 480-candidate] PSUM tile, then one DVE segmented reduce_max
  [128, 30, 16] -> [128, 30] per tile straight off PSUM (fp16 out).
Per core, per query: 240 chunk maxima; 8 cores -> 1920 chunks of 16.
"""

import os
import numpy as np
import ml_dtypes

M_, N_, D_, T_ = 2, 30000, 256, 3000
KD = M_ * D_                   # 512 contraction (data) dims
NCORES = 8
NSHARD = N_ // NCORES          # 3750
GW = 16                        # candidates per chunk (reduce_max group)
FCH = 480                      # candidate tile width (one PSUM bank, 30 groups)
NFC = 8                        # candidate tiles per core
NPAD = FCH * NFC               # 3840
NGRP = FCH // GW               # 30 chunk maxima per tile
ND = 510                       # data dims used for selection (2 stolen for bias)
S1 = 32.0                      # bias row 1 scale (query-side value)
CENTER = 512.0                 # |y|^2 centering (cancels in ranking)
QT = 128                       # queries per tile (PSUM partition dim)
NQ = 6016                      # 6000 queries padded to 47 tiles
NQT = NQ // QT                 # 47
QBLK = 4                       # query tiles per DMA block
NBLK = 12                      # 11 full + one 3-tile block
NSEL = 48                      # chunks rescored per query on host

_cache = {}


def _build_program():
    import concourse.bass as bass
    import concourse.tile as tile
    from concourse import bacc, mybir

    dt = mybir.dt
    nc = bacc.Bacc(
        "TRN2", target_bir_lowering=False, debug=False, num_devices=NCORES
    )

    xq_d = nc.dram_tensor("xq", [128, 4, NQ], dt.float8e4, kind="ExternalInput")
    xs_d = nc.dram_tensor("xs", [128, 4, NPAD], dt.float8e4, kind="ExternalInput")
    cand_d = nc.dram_tensor("cand", [NBLK, 128, QBLK * NFC * NGRP], dt.float16,
                            kind="ExternalOutput")

    with tile.TileContext(nc) as tc:
        with (
            tc.tile_pool(name="resident", bufs=1) as res_pool,
            tc.tile_pool(name="xq", bufs=2) as xq_pool,
            tc.tile_pool(name="cand", bufs=2) as cand_pool,
            tc.tile_pool(name="psum", bufs=8, space=bass.MemorySpace.PSUM) as psum_pool,
        ):
            xs_sb = res_pool.tile([128, 4, NPAD], dt.float8e4, tag="xs")
            nc.sync.dma_start(out=xs_sb[:, :, :], in_=xs_d[:, :, :])

            for blk in range(NBLK):
                q0 = blk * QBLK * QT
                nqt = min(QBLK, NQT - blk * QBLK)
                xq_sb = xq_pool.tile([128, 4, nqt * QT], dt.float8e4, tag="xq",
                                     name="xq_sb")
                nc.sync.dma_start(out=xq_sb[:, :, :],
                                  in_=xq_d[:, :, q0:q0 + nqt * QT])
                cand_sb = cand_pool.tile([128, nqt * NFC * NGRP], dt.float16,
                                         tag="cand")
                for j in range(nqt):
                    for f in range(NFC):
                        ps = psum_pool.tile([128, NGRP, GW], dt.float32, tag="ps")
                        for kc in range(2):
                            nc.tensor.matmul(
                                ps[:, :, :],
                                lhsT=xq_sb[:, 2 * kc:2 * kc + 2,
                                           j * QT:(j + 1) * QT],
                                rhs=xs_sb[:, 2 * kc:2 * kc + 2,
                                          f * FCH:(f + 1) * FCH],
                                start=(kc == 0),
                                stop=(kc == 1),
                                perf_mode=mybir.MatmulPerfMode.DoubleRow,
                            )
                        o = (j * NFC + f) * NGRP
                        nc.vector.tensor_reduce(
                            cand_sb[:, o:o + NGRP], ps[:, :, :],
                            axis=mybir.AxisListType.X, op=mybir.AluOpType.max,
                        )
                nc.sync.dma_start(out=cand_d[blk, :, :nqt * NFC * NGRP],
                                  in_=cand_sb[:, :])

    nc.compile()
    return nc


def _get_program():
    if "nc" not in _cache:
        _cache["nc"] = _build_program()
    return _cache["nc"]


def _f8(a):
    return np.clip(np.asarray(a, np.float32), -240, 240).astype(
        ml_dtypes.float8_e4m3)


def _prep_inputs(X, q_idx):
    """X: [N, 512] fp32; q_idx: [NQ] int64. Returns per-core input maps."""
    # queries (shared): [NQ, 512] = 2x data dims then the two bias-row consts
    Qm = np.zeros((NQ, KD), np.float32)
    Qm[:2 * T_, :ND] = 2.0 * X[q_idx[:2 * T_], :ND]
    Qm[:2 * T_, ND] = S1
    Qm[:2 * T_, ND + 1] = 1.0
    xq = np.ascontiguousarray(
        _f8(Qm).reshape(NQ, 4, 128).transpose(2, 1, 0))       # [128, 4, NQ]

    sqy = (X.astype(np.float64) ** 2).sum(1).astype(np.float32)
    bias_t = -(sqy - CENTER)                                   # ~ +-150
    b1 = _f8(bias_t / S1).astype(np.float32)
    b2 = _f8(bias_t - S1 * b1).astype(np.float32)

    per_core = []
    for ci in range(NCORES):
        sl = slice(ci * NSHARD, (ci + 1) * NSHARD)
        Z = np.zeros((NPAD, KD), np.float32)
        Z[:NSHARD, :ND] = X[sl, :ND]
        Z[:NSHARD, ND] = b1[sl]
        Z[:NSHARD, ND + 1] = b2[sl]
        Z[NSHARD:, ND:] = -240.0          # pad candidates rank last (~ -7920)
        xs = np.ascontiguousarray(
            _f8(Z).reshape(NPAD, 4, 128).transpose(2, 1, 0))  # [128, 4, NPAD]
        per_core.append({"xq": xq, "xs": xs})
    return per_core


def _mine_chunkmax(in_maps, trace=False):
    from concourse.bass_utils import run_bass_kernel_spmd

    nc = _get_program()
    try:
        res = run_bass_kernel_spmd(nc, in_maps, list(range(NCORES)), trace=trace)
    except Exception:
        if not trace:
            raise
        res = run_bass_kernel_spmd(nc, in_maps, list(range(NCORES)), trace=False)
    _cache["last_result"] = res
    cores = []
    for i in range(NCORES):
        c = res.results[i]["cand"]                 # [NBLK, 128, QBLK*240]
        c = c.reshape(NBLK, 128, QBLK, NFC * NGRP).transpose(0, 2, 1, 3)
        cores.append(c.reshape(NBLK * QBLK * 128, NFC * NGRP)[:NQ])
    return np.concatenate(cores, axis=1)           # [NQ, 1920]


def kernel(outlayer, c, train_ill, k):
    k = int(k)
    outlayer = np.asarray(outlayer, np.float32)
    train_ill = np.asarray(train_ill)
    X = np.ascontiguousarray(
        outlayer.transpose(1, 0, 2).reshape(N_, KD)).astype(np.float32)
    left = train_ill[:, 0].astype(np.int64)
    right = train_ill[:, 1].astype(np.int64)
    q_idx = np.concatenate([right, left, np.zeros(NQ - 2 * T_, np.int64)])

    in_maps = _prep_inputs(X, q_idx)
    cm = _mine_chunkmax(
        in_maps, trace=bool(int(os.environ.get("KNN_TRACE", "0"))))
    cm = cm.astype(np.float32)

    # top-NSEL chunks per query -> candidate lists with known indices
    top_chunks = np.argpartition(-cm[:2 * T_], NSEL, axis=1)[:, :NSEL]
    core = top_chunks // (NPAD // GW)
    jj = top_chunks % (NPAD // GW)
    base = core * NSHARD + jj * GW
    cand = base[:, :, None] + np.arange(GW)[None, None, :]     # [2T, NSEL, 16]
    valid = (jj[:, :, None] * GW + np.arange(GW)[None, None, :]) < NSHARD
    cand = np.where(valid, cand, 0).reshape(2 * T_, NSEL * GW)
    valid = valid.reshape(2 * T_, NSEL * GW)

    # exact rescore (fp32 gather/dot, fp64 assembly)
    nkeep = k + 1
    sq64 = (X.astype(np.float64) ** 2).sum(1)
    B_all = np.zeros((2 * T_, nkeep))
    for q0 in range(0, 2 * T_, 256):
        q1 = min(q0 + 256, 2 * T_)
        qv = X[q_idx[q0:q1]]                                   # [B, 512]
        cv = X[cand[q0:q1]]                                    # [B, C, 512]
        dot = np.matmul(cv, qv[:, :, None].astype(np.float32))[:, :, 0]
        d = (sq64[q_idx[q0:q1], None] + sq64[cand[q0:q1]]
             - 2.0 * dot.astype(np.float64))
        d = np.where(valid[q0:q1], d, np.inf)
        idx = np.argpartition(d, nkeep, axis=1)[:, :nkeep]
        # exact fp64 distances for the kept few
        g = X.astype(np.float64)[np.take_along_axis(cand[q0:q1], idx, axis=1)]
        dd = ((qv[:, None, :].astype(np.float64) - g) ** 2).sum(2)
        dd = np.where(np.take_along_axis(valid[q0:q1], idx, axis=1), dd, np.inf)
        B_all[q0:q1] = np.sort(dd, axis=1)
    B2 = B_all[:T_, 1:]            # right-query mining
    B1 = B_all[T_:, 1:]            # left-query mining

    X64 = X.astype(np.float64)
    D = ((X64[left] - X64[right]) ** 2).sum(1) + 1.0
    L1 = np.maximum(D[:, None] - B1, 0.0)
    L2 = np.maximum(D[:, None] - B2, 0.0)
    loss = (L1.mean() + L2.mean()) / 2.0
    return np.asarray(loss, dtype=np.float32)


# revision 4
# speedup vs baseline: 1.6384x; 1.1340x over previous
"""Trainium2 Bass kernel for CrossDecoder kNN-mining margin loss (fp8, v2).

Mining strategy (vs 336us fp16 baseline): PE work halves via fp8 E4M3
perf_mode=DoubleRow matmuls (2 fp8 weights/cell, K=256/matmul).  fp8 scores
are too noisy (sigma~3.3) to use as distance values, so the device returns
per-16-candidate-chunk maxima; the host selects top chunks per query (the
chunk POSITION identifies its candidates), rescores those exactly, and
rebuilds the exact top-(k+1) distances.  Chunk selection has ~5 sigma of
noise margin at NSEL=48.

score(q,j) = sum_d 2 q_d y_jd   (510 of 512 data dims, fp8)
           + 32*b1_j + b2_j     (2 fp8 bias rows ~= -(|y_j|^2 - 512))

v2 device-side changes (from 235us v1):
  - kc-outer loop: 8 consecutive matmuls share the same stationary operand
    (query tile, one K-half), so LDWEIGHTS can be elided/overlapped instead
    of serializing ~135ns per matmul.
  - the PSUM scan (the DVE wall: fp32 PSUM reads are 1 elem/cycle @
    0.96GHz, tensor_reduce is 1x-only) is split across engines: DVE
    reduce_max's banks 0..3 directly (two 2-bank [128,2,30,16] reduces,
    amortizing the ~120cyc init), while ScalarE (otherwise idle, own PSUM
    port) copies banks 4..7 to SBUF fp16 and DVE finishes those with a 2x
    tensor_tensor max tree + tiny reduce.
"""

import os
import numpy as np
import ml_dtypes

M_, N_, D_, T_ = 2, 30000, 256, 3000
KD = M_ * D_                   # 512 contraction (data) dims
NCORES = 8
NSHARD = N_ // NCORES          # 3750
GW = 16                        # candidates per chunk (reduce_max group)
FCH = 480                      # candidate tile width (one PSUM bank, 30 groups)
NFC = 8                        # candidate tiles per core
NPAD = FCH * NFC               # 3840
NGRP = FCH // GW               # 30 chunk maxima per tile
ND = 510                       # data dims used for selection (2 stolen for bias)
S1 = 32.0                      # bias row 1 scale (query-side value)
CENTER = 512.0                 # |y|^2 centering (cancels in ranking)
QT = 128                       # queries per tile (PSUM partition dim)
NQ = 6016                      # 6000 queries padded to 47 tiles
NQT = NQ // QT                 # 47
QBLK = 4                       # query tiles per DMA block
NBLK = 12                      # 11 full + one 3-tile block
NSEL = 48                      # chunks rescored per query on host

_cache = {}


def _build_program():
    import concourse.bass as bass
    import concourse.tile as tile
    from concourse import bacc, mybir

    dt = mybir.dt
    nc = bacc.Bacc(
        "TRN2", target_bir_lowering=False, debug=False, num_devices=NCORES
    )

    xq_d = nc.dram_tensor("xq", [128, 4, NQ], dt.float8e4, kind="ExternalInput")
    xs_d = nc.dram_tensor("xs", [128, 4, NPAD], dt.float8e4, kind="ExternalInput")
    cand_d = nc.dram_tensor("cand", [NBLK, 128, QBLK * NFC * NGRP], dt.float16,
                            kind="ExternalOutput")

    DR = mybir.MatmulPerfMode.DoubleRow

    with tile.TileContext(nc) as tc:
        with (
            tc.tile_pool(name="resident", bufs=1) as res_pool,
            tc.tile_pool(name="xq", bufs=2) as xq_pool,
            tc.tile_pool(name="cand", bufs=2) as cand_pool,
            tc.tile_pool(name="scr", bufs=2) as scr_pool,
            tc.tile_pool(name="psum", bufs=4, space=bass.MemorySpace.PSUM) as psum_pool,
        ):
            xs_sb = res_pool.tile([128, 4, NPAD], dt.float8e4, tag="xs")
            nc.sync.dma_start(out=xs_sb[:, :, :], in_=xs_d[:, :, :])

            for blk in range(NBLK):
                q0 = blk * QBLK * QT
                nqt = min(QBLK, NQT - blk * QBLK)
                xq_sb = xq_pool.tile([128, 4, nqt * QT], dt.float8e4, tag="xq",
                                     name="xq_sb")
                nc.sync.dma_start(out=xq_sb[:, :, :],
                                  in_=xq_d[:, :, q0:q0 + nqt * QT])
                cand_sb = cand_pool.tile([128, nqt, NFC, NGRP], dt.float16,
                                         tag="cand")
                for j in range(nqt):
                    # 4 PSUM tiles of 2 banks each; bank h of pair p holds
                    # candidate chunk f = 2p+h.
                    ps = [psum_pool.tile([128, 2, NGRP, GW], dt.float32,
                                         tag="ps", name=f"ps{p}",
                                         padded_shape=[None, None, 32, None])
                          for p in range(4)]
                    for kc in range(2):
                        for p in range(4):
                            for h in range(2):
                                f = 2 * p + h
                                nc.tensor.matmul(
                                    ps[p][:, h, :, :],
                                    lhsT=xq_sb[:, 2 * kc:2 * kc + 2,
                                               j * QT:(j + 1) * QT],
                                    rhs=xs_sb[:, 2 * kc:2 * kc + 2,
                                              f * FCH:(f + 1) * FCH],
                                    start=(kc == 0),
                                    stop=(kc == 1),
                                    perf_mode=DR,
                                )
                    # banks 0..3 (pairs 0,1): direct DVE segmented reduce
                    for p in range(2):
                        nc.vector.tensor_reduce(
                            cand_sb[:, j, 2 * p:2 * p + 2, :], ps[p][:, :, :, :],
                            axis=mybir.AxisListType.X, op=mybir.AluOpType.max,
                        )
                    # banks 4..7 (pairs 2,3): ScalarE copies PSUM->SBUF fp16,
                    # then a DVE 2x tensor_tensor max tree + small reduce.
                    scr = scr_pool.tile([128, 4, NGRP, GW], dt.float16, tag="scr")
                    for p in range(2, 4):
                        nc.scalar.activation(
                            scr[:, 2 * (p - 2):2 * (p - 2) + 2, :, :],
                            ps[p][:, :, :, :],
                            mybir.ActivationFunctionType.Copy,
                        )
                    t1 = scr_pool.tile([128, 4, NGRP, 8], dt.float16, tag="t1")
                    nc.vector.tensor_tensor(
                        t1[:, :, :, :], scr[:, :, :, 0:8], scr[:, :, :, 8:16],
                        mybir.AluOpType.max)
                    t2 = scr_pool.tile([128, 4, NGRP, 4], dt.float16, tag="t2")
                    nc.vector.tensor_tensor(
                        t2[:, :, :, :], t1[:, :, :, 0:4], t1[:, :, :, 4:8],
                        mybir.AluOpType.max)
                    t3 = scr_pool.tile([128, 4, NGRP, 2], dt.float16, tag="t3")
                    nc.vector.tensor_tensor(
                        t3[:, :, :, :], t2[:, :, :, 0:2], t2[:, :, :, 2:4],
                        mybir.AluOpType.max)
                    nc.vector.tensor_reduce(
                        cand_sb[:, j, 4:8, :], t3[:, :, :, :],
                        axis=mybir.AxisListType.X, op=mybir.AluOpType.max,
                    )
                nc.sync.dma_start(out=cand_d[blk, :, :nqt * NFC * NGRP],
                                  in_=cand_sb[:, :, :, :])

    nc.compile()
    return nc


def _get_program():
    if "nc" not in _cache:
        _cache["nc"] = _build_program()
    return _cache["nc"]


def _f8(a):
    return np.clip(np.asarray(a, np.float32), -240, 240).astype(
        ml_dtypes.float8_e4m3)


def _prep_inputs(X, q_idx):
    """X: [N, 512] fp32; q_idx: [NQ] int64. Returns per-core input maps."""
    # queries (shared): [NQ, 512] = 2x data dims then the two bias-row consts
    Qm = np.zeros((NQ, KD), np.float32)
    Qm[:2 * T_, :ND] = 2.0 * X[q_idx[:2 * T_], :ND]
    Qm[:2 * T_, ND] = S1
    Qm[:2 * T_, ND + 1] = 1.0
    xq = np.ascontiguousarray(
        _f8(Qm).reshape(NQ, 4, 128).transpose(2, 1, 0))       # [128, 4, NQ]

    sqy = (X.astype(np.float64) ** 2).sum(1).astype(np.float32)
    bias_t = -(sqy - CENTER)                                   # ~ +-150
    b1 = _f8(bias_t / S1).astype(np.float32)
    b2 = _f8(bias_t - S1 * b1).astype(np.float32)

    per_core = []
    for ci in range(NCORES):
        sl = slice(ci * NSHARD, (ci + 1) * NSHARD)
        Z = np.zeros((NPAD, KD), np.float32)
        Z[:NSHARD, :ND] = X[sl, :ND]
        Z[:NSHARD, ND] = b1[sl]
        Z[:NSHARD, ND + 1] = b2[sl]
        Z[NSHARD:, ND:] = -240.0          # pad candidates rank last (~ -7920)
        xs = np.ascontiguousarray(
            _f8(Z).reshape(NPAD, 4, 128).transpose(2, 1, 0))  # [128, 4, NPAD]
        per_core.append({"xq": xq, "xs": xs})
    return per_core


def _mine_chunkmax(in_maps, trace=False):
    from concourse.bass_utils import run_bass_kernel_spmd

    nc = _get_program()
    try:
        res = run_bass_kernel_spmd(nc, in_maps, list(range(NCORES)), trace=trace)
    except Exception:
        if not trace:
            raise
        res = run_bass_kernel_spmd(nc, in_maps, list(range(NCORES)), trace=False)
    _cache["last_result"] = res
    cores = []
    for i in range(NCORES):
        c = res.results[i]["cand"]                 # [NBLK, 128, QBLK*240]
        c = c.reshape(NBLK, 128, QBLK, NFC * NGRP).transpose(0, 2, 1, 3)
        cores.append(c.reshape(NBLK * QBLK * 128, NFC * NGRP)[:NQ])
    return np.concatenate(cores, axis=1)           # [NQ, 1920]


def kernel(outlayer, c, train_ill, k):
    k = int(k)
    outlayer = np.asarray(outlayer, np.float32)
    train_ill = np.asarray(train_ill)
    X = np.ascontiguousarray(
        outlayer.transpose(1, 0, 2).reshape(N_, KD)).astype(np.float32)
    left = train_ill[:, 0].astype(np.int64)
    right = train_ill[:, 1].astype(np.int64)
    q_idx = np.concatenate([right, left, np.zeros(NQ - 2 * T_, np.int64)])

    in_maps = _prep_inputs(X, q_idx)
    cm = _mine_chunkmax(
        in_maps, trace=bool(int(os.environ.get("KNN_TRACE", "0"))))
    cm = cm.astype(np.float32)

    # top-NSEL chunks per query -> candidate lists with known indices
    top_chunks = np.argpartition(-cm[:2 * T_], NSEL, axis=1)[:, :NSEL]
    core = top_chunks // (NPAD // GW)
    jj = top_chunks % (NPAD // GW)
    base = core * NSHARD + jj * GW
    cand = base[:, :, None] + np.arange(GW)[None, None, :]     # [2T, NSEL, 16]
    valid = (jj[:, :, None] * GW + np.arange(GW)[None, None, :]) < NSHARD
    cand = np.where(valid, cand, 0).reshape(2 * T_, NSEL * GW)
    valid = valid.reshape(2 * T_, NSEL * GW)

    # exact rescore (fp32 gather/dot, fp64 assembly)
    nkeep = k + 1
    sq64 = (X.astype(np.float64) ** 2).sum(1)
    B_all = np.zeros((2 * T_, nkeep))
    for q0 in range(0, 2 * T_, 256):
        q1 = min(q0 + 256, 2 * T_)
        qv = X[q_idx[q0:q1]]                                   # [B, 512]
        cv = X[cand[q0:q1]]                                    # [B, C, 512]
        dot = np.matmul(cv, qv[:, :, None].astype(np.float32))[:, :, 0]
        d = (sq64[q_idx[q0:q1], None] + sq64[cand[q0:q1]]
             - 2.0 * dot.astype(np.float64))
        d = np.where(valid[q0:q1], d, np.inf)
        idx = np.argpartition(d, nkeep, axis=1)[:, :nkeep]
        # exact fp64 distances for the kept few
        g = X.astype(np.float64)[np.take_along_axis(cand[q0:q1], idx, axis=1)]
        dd = ((qv[:, None, :].astype(np.float64) - g) ** 2).sum(2)
        dd = np.where(np.take_along_axis(valid[q0:q1], idx, axis=1), dd, np.inf)
        B_all[q0:q1] = np.sort(dd, axis=1)
    B2 = B_all[:T_, 1:]            # right-query mining
    B1 = B_all[T_:, 1:]            # left-query mining

    X64 = X.astype(np.float64)
    D = ((X64[left] - X64[right]) ** 2).sum(1) + 1.0
    L1 = np.maximum(D[:, None] - B1, 0.0)
    L2 = np.maximum(D[:, None] - B2, 0.0)
    loss = (L1.mean() + L2.mean()) / 2.0
    return np.asarray(loss, dtype=np.float32)
